# revision 49
# baseline (speedup 1.0000x reference)
"""CAGroup3D head kernel for 8 Trainium2 NeuronCores (data-parallel over voxels).

v2 — minimizes host<->device staging (the dominant cost of a dispatch through
the axon tunnel) and the on-device body time.

Staging strategy
----------------
- Per core only its own feats slice [NL, C] bf16 (2.1MB) and a compact int16
  neighbor-index tensor (0.47MB) are staged per call. The full 65536-row
  gather table is assembled ON DEVICE with an 8-core AllGather collective.
- All weight stacks are baked into the NEFF as inline Const tensors (DMA'd to
  HBM once at model load, not per dispatch). The program cache is keyed by a
  content hash of the weights, so different weights rebuild correctly.
- Outputs are written in bf16 ([17, cols] blocks) and upcast on host; the
  donated output buffers are created device-side (jnp.zeros under jit), so no
  zero bytes cross the tunnel.

Body strategy
-------------
- Single-range neighbor gather: indices are encoded as (row - 32768) int16 and
  the gather's in_ap points at row 32768 of the table. The Q7 DGE ucode
  sign-extends idx and computes addr = base + idx*stride with a
  multiply-unsigned-signed accumulate, so negative indices correctly address
  the lower half. This removes the old 3-range redundancy (3x gather bytes and
  3x nconv matmuls). Trailing-negative indices are trimmed by the ucode, so
  every gather is padded to 2176 indices with a final all-zero (valid) group.
- featsT is produced on device via the DMA crossbar transpose.
- The per-class semantic mask lives transposed [10, NL]; the head matmul
  produces [17, cols] blocks directly (stationary = head weights, reg columns
  first so the ACT Exp starts at partition 0), and one fused
  scalar_tensor_tensor applies cls bias + mask and casts to bf16.
- ELU = max(x, min(exp(x),1)-1): Exp on ACT, min/add + max on DVE. Pool (Q7)
  elementwise is ~6x slower than the cost model claims on HW and GPSIMD
  cannot read PSUM at all, so Pool runs only the gathers. Every second L4 ELU
  uses the (relu, expm1) two-stream form (no merge; the head matmul
  accumulates both streams) to shift DVE load onto ACT/PE.
- Chain PSUM rotates 3 slots during the gather phase and borrows the idle
  nconv accumulator slot as a 4th during the voted phase.
"""

import sys

sys.path.insert(0, "/opt/trn_rl_repo")

import hashlib
import numpy as np
import ml_dtypes

import concourse.bass as bass  # noqa: F401
import concourse.mybir as mybir
from concourse import tile, bacc

bf16 = ml_dtypes.bfloat16
F32 = mybir.dt.float32
BF16 = mybir.dt.bfloat16
I16 = mybir.dt.int16
AF = mybir.ActivationFunctionType
ALU = mybir.AluOpType

N = 65536
C = 128
NCLS = 10
K = 27
NCORES = 8
NL = N // NCORES            # 8192 voxels per core
M = 2 * NL                  # 16384 chain rows per core
T = 1024                    # nconv chunk cols
TC = 1024                   # chain sub-chunk cols
NCHUNK_N = NL // T          # 8 nconv chunks
NCH = M // TC               # 16 chain sub-chunks
NH = NL // TC               # 8 per half
HALF = 32768
TPAD = T + 128              # gather idx count (pad group keeps last idx >= 0)
IW = TPAD // 16             # 136 idx free-dim per (chunk, tap)
IDXF = NCHUNK_N * K * IW
THR_LOGIT = float(np.log(0.15 / 0.85))

# engine-balance knobs. Pool (GPSIMD) cannot touch PSUM on HW, so the ELU
# merge (max with the PSUM preact) and head stt always run on DVE; Pool can
# only take the SBUF-only min/add (ts) ops. SPLIT_MOD converts every n-th
# chain ELU to the (relu, expm1) two-stream form: no merge at all, one extra
# ACT op and one extra matmul stream on the consumer (PE has headroom).
TS_POOL_MOD = 0             # Pool elementwise is ~6x slower than modeled on HW: keep 0
SPLIT_MOD = 2               # every n-th L4 ELU uses the two-stream form
ILEAVE = 6                  # chain units interleaved at once
EM_BUFS = 4                 # bufs for e/m ELU temporaries
Y_BUFS = 6                  # bufs for layer outputs (y1 lives until L3)
MR_BUFS = 6                 # bufs for head mask/stage tiles
PSC_BUFS = 3                # chain PSUM slots
GT_BUFS = 6                 # gather destination tiles
PRE_CHAINS = 1              # feats-half units emitted before first gather

LAST_EXEC_NS = None
LAST_RESULTS = None
_PROGRAM = {}


def _build_program(weights, use_bias: bool, reps: int = 1):
    nc = bacc.Bacc(None, target_bir_lowering=False, debug=False,
                   num_swdge_queues=4, num_devices=NCORES)

    fsl = nc.declare_dram_parameter("fsl", [NL, C], BF16, isOutput=False)
    idxw = nc.declare_dram_parameter("idxw", [16, IDXF], I16, isOutput=False)
    outd = nc.declare_dram_parameter("out", [NCLS, NCH, 17, TC], BF16, isOutput=True)

    wsem_c = nc.inline_tensor(weights["wsem"], name="wsem_c")
    fo_c = nc.inline_tensor(weights["fo"], name="fo_c")
    wcls_c = nc.inline_tensor(weights["wcls"], name="wcls_c")
    hd_c = nc.inline_tensor(weights["hd"], name="hd_c")
    thr_c = nc.inline_tensor(weights["thr"], name="thr_c")
    hbias_c = nc.inline_tensor(weights["hbias"], name="hbias_c")
    if use_bias:
        lbias_c = nc.inline_tensor(weights["lbias"], name="lbias_c")

    fs_b = nc.dram_tensor("fs_b", [NL, C], BF16)
    tab = nc.dram_tensor("tab", [N, C], BF16)
    maskD = nc.dram_tensor("maskD", [NCLS, NL], BF16)

    tctr = [0]
    sctr = [0]
    gather_phase = [True]   # True while nconv gathers share the Pool queue

    def ts_eng():
        # Pool (Q7) elementwise is only viable when strictly separated from
        # gathers: interleaving forces library reloads (~ms each on HW). Never
        # route Pool work during the gather phase.
        if TS_POOL_MOD == 0 or gather_phase[0]:
            return nc.vector
        tctr[0] += 1
        return nc.gpsimd if (tctr[0] % TS_POOL_MOD == 0) else nc.vector

    def want_split():
        if SPLIT_MOD == 0:
            return False
        sctr[0] += 1
        return sctr[0] % SPLIT_MOD == 0

    pctr = [0]

    with tile.TileContext(nc) as tc:
        with tc.tile_pool(name="const", bufs=1) as cp, \
             tc.tile_pool(name="work", bufs=1) as wp, \
             tc.tile_pool(name="ps", bufs=PSC_BUFS, space="PSUM") as pp:

            # ---------------- prologue: table + residents ----------------
            # The DMA transpose must be emitted BEFORE the collective: the
            # scheduler serializes DmaTransposeAnt against collectives, and
            # the whole chain pipeline hangs off fT.
            fT = cp.tile([C, NL], BF16, name="fT")
            nc.sync.dma_start_transpose(fT[:], fsl[:])

            nc.sync.dma_start(out=fs_b[:], in_=fsl[:])
            nc.gpsimd.collective_compute(
                "AllGather", ALU.bypass,
                replica_groups=[list(range(NCORES))],
                ins=[fs_b[:].opt()], outs=[tab[:].opt()])

            wsem_sb = cp.tile([C, NCLS], BF16)
            nc.sync.dma_start(out=wsem_sb[:], in_=wsem_c[:])
            fo_sb = cp.tile([C, K * C], BF16)
            nc.sync.dma_start(out=fo_sb[:], in_=fo_c[:])
            wcls_sb = cp.tile([C, NCLS * 5 * C], BF16)
            nc.sync.dma_start(out=wcls_sb[:], in_=wcls_c[:])
            hd_sb = cp.tile([C, NCLS * 17], BF16)
            nc.sync.dma_start(out=hd_sb[:], in_=hd_c[:])
            thr_sb = cp.tile([NCLS, 1], F32)
            nc.sync.dma_start(out=thr_sb[:], in_=thr_c[:])
            hbias_sb = cp.tile([17, 1], F32)
            nc.sync.dma_start(out=hbias_sb[:], in_=hbias_c[:])
            if use_bias:
                lbias_sb = cp.tile([C, NCLS * 4 + 1], F32)
                nc.sync.dma_start(out=lbias_sb[:], in_=lbias_c[:])

            oft = cp.tile([C, NL], BF16, name="oft")      # offset_features^T

            def W(cls, which):
                return wcls_sb[:, (cls * 5 + which) * C:(cls * 5 + which + 1) * C]

            def lbias(cls, layer):
                if not use_bias:
                    return None
                col = NCLS * 4 if cls is None else cls * 4 + layer
                return lbias_sb[:, col:col + 1]

            # ---------------- sem + transposed mask ----------------
            maskT = cp.tile([NCLS, NL], BF16, name="maskT")
            for b in range(NL // 512):
                pss = pp.tile([NCLS, 512], F32, tag="psc", name="pss")
                nc.tensor.matmul(pss[:], wsem_sb[:], fT[:, b * 512:(b + 1) * 512],
                                 start=True, stop=True)
                nc.vector.tensor_scalar(maskT[:, b * 512:(b + 1) * 512],
                                        pss[:], thr_sb[:, 0:1], None, ALU.is_gt)
            nc.sync.dma_start(out=maskD[:], in_=maskT[:])

            # ---------------- pipeline pieces ----------------
            def elu(ps_, into, bias_ap=None, w=TC, tag="", may_split=False):
                """Returns a list of stream APs summing to ELU(ps+bias).

                into: destination AP ([C, w]) to write a merged result, or
                None to allocate a tile. may_split allows the two-stream
                (relu, min(exp,1)-1) form (accumulated by the consumer)."""
                sfx = "n" if w != TC else ""
                e = wp.tile([C, w], BF16, tag="e" + sfx, bufs=EM_BUFS, name="e" + tag)
                if bias_ap is None:
                    nc.scalar.activation(e[:], ps_[:], AF.Exp)
                else:
                    nc.scalar.activation(e[:], ps_[:], AF.Exp, bias=bias_ap)
                m = wp.tile([C, w], BF16, tag="m" + sfx, bufs=EM_BUFS, name="m" + tag)
                split = may_split and into is None and bias_ap is None and want_split()
                if split or bias_ap is not None:
                    # materialized min(e,1)-1 (stream form / bias-merge form)
                    ts_eng().tensor_scalar(m[:], e[:], 1.0, -1.0, ALU.min, ALU.add)
                else:
                    # single-ALU min keeps DVE in its fast mode; the -1 folds
                    # into the merge's scalar slot below.
                    ts_eng().tensor_scalar(m[:], e[:], 1.0, None, ALU.min)
                if split:
                    r = wp.tile([C, w], BF16, tag="r" + tag, bufs=EM_BUFS,
                                name="r" + tag)
                    nc.scalar.activation(r[:], ps_[:], AF.Relu)
                    return [r[:], m[:]]
                if into is None:
                    y = wp.tile([C, w], BF16, tag="y" + tag, bufs=Y_BUFS, name="y" + tag)
                    into = y[:]
                if bias_ap is None:
                    nc.vector.scalar_tensor_tensor(into, m[:], -1.0, ps_[:],
                                                   ALU.add, ALU.max)
                else:
                    nc.vector.scalar_tensor_tensor(into, ps_[:], bias_ap, m[:],
                                                   ALU.add, ALU.max)
                return [into]

            def layer(ps_, wstreams):
                """wstreams: list of (W_ap, [x_ap, ...]) pairs."""
                flat = [(w_ap, x) for w_ap, xs in wstreams for x in xs]
                nstream = len(flat)
                for h in range(TC // 512):
                    sl = slice(h * 512, (h + 1) * 512)
                    for i, (w_ap, x_ap) in enumerate(flat):
                        nc.tensor.matmul(ps_[:, sl], w_ap, x_ap[:, sl],
                                         start=(i == 0), stop=(i == nstream - 1))

            def nconv_chunk(n):
                idx_sb = wp.tile([128, K * IW], I16, tag="idx", bufs=2, name="idx")
                for g in range(8):
                    nc.sync.dma_start(
                        out=idx_sb[16 * g:16 * (g + 1), :],
                        in_=idxw[:, n * K * IW:(n + 1) * K * IW])
                ps = pp.tile([C, T], F32, tag="psn", bufs=1, name="nconv_ps")
                for k in range(K):
                    gt = wp.tile([C, 1, TPAD], BF16, tag="gt", bufs=GT_BUFS, name="gt")
                    nc.gpsimd.dma_gather(
                        out_ap=gt[:],
                        in_ap=tab[HALF:N, :],
                        idxs_ap=idx_sb[:, k * IW:(k + 1) * IW],
                        num_idxs=TPAD,
                        num_idxs_reg=TPAD,
                        elem_size=C,
                        transpose=True,
                        single_packet=False,
                        queue_num=k % 4,
                    )
                    for h in range(T // 512):
                        nc.tensor.matmul(ps[:, h * 512:(h + 1) * 512],
                                         fo_sb[:, k * C:(k + 1) * C],
                                         gt[:, 0, h * 512:(h + 1) * 512],
                                         start=(k == 0), stop=(k == K - 1))
                elu(ps, oft[:, n * T:(n + 1) * T], bias_ap=lbias(None, 0), w=T)

            def chain_ps(shape, name):
                # during the voted phase the nconv accumulator slot (tag psn)
                # is idle — rotate it in as a 4th chain PSUM slot.
                if not gather_phase[0]:
                    pctr[0] += 1
                    if pctr[0] % 4 == 0:
                        return pp.tile(shape, F32, tag="psn", bufs=1, name=name)
                return pp.tile(shape, F32, tag="psc", name=name)

            def chain_unit_gen(cchunk, cls):
                x = oft if cchunk < NH else fT
                base = (cchunk % NH) * TC
                xs = x[:, base:base + TC]

                ps1 = chain_ps([C, TC], "ps1")
                layer(ps1, [(W(cls, 0), [xs])])
                y1 = elu(ps1, None, lbias(cls, 0), tag="1")
                yield

                ps2 = chain_ps([C, TC], "ps2")
                layer(ps2, [(W(cls, 1), y1)])
                y2 = elu(ps2, None, lbias(cls, 1), tag="2")
                yield

                ps3 = chain_ps([C, TC], "ps3")
                layer(ps3, [(W(cls, 2), y1), (W(cls, 3), y2)])
                y3 = elu(ps3, None, lbias(cls, 2), tag="3")
                yield

                ps4 = chain_ps([C, TC], "ps4")
                layer(ps4, [(W(cls, 4), y3)])
                y4 = elu(ps4, None, lbias(cls, 3), tag="4", may_split=True)
                yield

                hp = chain_ps([17, TC], "hp")
                for h in range(TC // 512):
                    sl = slice(h * 512, (h + 1) * 512)
                    for j, s in enumerate(y4):
                        nc.tensor.matmul(hp[:, sl],
                                         hd_sb[:, cls * 17:(cls + 1) * 17],
                                         s[:, sl], start=(j == 0),
                                         stop=(j == len(y4) - 1))
                nc.scalar.activation(hp[0:6, :], hp[0:6, :], AF.Exp)
                mrep = wp.tile([17, TC], BF16, tag="mr", bufs=MR_BUFS, name="mrep")
                nc.sync.dma_start(
                    out=mrep[:],
                    in_=maskD[cls:cls + 1, base:base + TC].broadcast_to([17, TC]))
                stage = wp.tile([17, TC], BF16, tag="st", bufs=MR_BUFS, name="stage")
                nc.vector.scalar_tensor_tensor(
                    stage[:], hp[:], hbias_sb[:, 0:1], mrep[:],
                    ALU.add, ALU.mult)
                nc.sync.dma_start(out=outd[cls, cchunk], in_=stage[:])

            def chain_units(cchunk):
                for c0 in range(0, NCLS, ILEAVE):
                    gens = [chain_unit_gen(cchunk, c0 + d)
                            for d in range(min(ILEAVE, NCLS - c0))]
                    done = False
                    while not done:
                        done = True
                        for gx in gens:
                            try:
                                next(gx)
                                done = False
                            except StopIteration:
                                pass

            # ---------------- emission ----------------
            # feats-half chains first (they don't touch the gather table, so
            # they hide the AllGather latency); nconv chunks interleaved with
            # the remaining feats-half units AND voted-half units (lagging
            # their oft chunk by 2) so gather DMA spreads across the whole
            # timeline instead of bunching in the first half.
            for _rep in range(reps):
                gather_phase[0] = True
                for s in range(PRE_CHAINS):
                    chain_units(NH + s)
                vq = 0
                for n in range(NCHUNK_N):
                    nconv_chunk(n)
                    if PRE_CHAINS + n < NH:
                        chain_units(NH + PRE_CHAINS + n)
                    if n >= 2 and vq <= n - 2:
                        chain_units(vq)
                        vq += 1
                gather_phase[0] = False
                while vq < NH:
                    chain_units(vq)
                    vq += 1

    nc.compile()
    return nc


def _prepare(feats, nbr, Wsem, bsem,
             fo_w, fo_g, fo_b,
             cls_out_w, cls_out_g, cls_out_b,
             up_w, up_g, up_b,
             fuse_w, fuse_g, fuse_b,
             exp_w, exp_g, exp_b,
             ctr_w, reg_w, cls_w, cls_b, scales):
    feats = np.asarray(feats, dtype=np.float32)
    nbr = np.asarray(nbr, dtype=np.int64)

    def fold(w, g):
        return (np.asarray(w, np.float32) * np.asarray(g, np.float32)[..., None, :]).astype(bf16)

    fo_bf = fold(fo_w, np.broadcast_to(np.asarray(fo_g, np.float32), (K, C)))
    fo_bf = np.ascontiguousarray(fo_bf.transpose(1, 0, 2).reshape(C, K * C))
    w1 = fold(cls_out_w, cls_out_g)
    w2 = fold(up_w, up_g)
    wf = fold(fuse_w, fuse_g)
    w4 = fold(exp_w, exp_g)
    wcls_np = np.stack([w1, w2, wf[:, :C, :], wf[:, C:, :], w4], axis=1)
    wcls_np = np.ascontiguousarray(wcls_np.transpose(2, 0, 1, 3).reshape(C, NCLS * 5 * C))

    # head column order on device: [reg(6), ctr(1), cls(10)] — the Exp must
    # start at partition 0. Host assembly permutes back to [ctr, reg, cls].
    hd_np = np.concatenate([
        np.asarray(scales, np.float32)[:, None, None] * np.asarray(reg_w, np.float32)[None],
        np.broadcast_to(np.asarray(ctr_w, np.float32), (NCLS, C, 1)),
        np.broadcast_to(np.asarray(cls_w, np.float32), (NCLS, C, NCLS)),
    ], axis=2).astype(bf16)
    hd_np = np.ascontiguousarray(hd_np.transpose(1, 0, 2).reshape(C, NCLS * 17))

    thr_np = (THR_LOGIT - np.asarray(bsem, np.float32)).reshape(NCLS, 1).astype(np.float32)
    hbias_np = np.concatenate([np.zeros(7, np.float32),
                               np.asarray(cls_b, np.float32)]).reshape(17, 1)

    biases = np.stack([
        np.asarray(cls_out_b, np.float32),
        np.asarray(up_b, np.float32),
        np.asarray(fuse_b, np.float32),
        np.asarray(exp_b, np.float32),
    ], axis=1)                                             # [NCLS, 4, C]
    fo_b_np = np.asarray(fo_b, np.float32)
    use_bias = bool(np.any(biases != 0.0) or np.any(fo_b_np != 0.0))
    lbias_np = np.concatenate(
        [biases.transpose(2, 0, 1).reshape(C, NCLS * 4),
         fo_b_np.reshape(C, 1)], axis=1).astype(np.float32)

    weights = {
        "wsem": np.asarray(Wsem, np.float32).astype(bf16),
        "fo": fo_bf,
        "wcls": wcls_np,
        "hd": hd_np,
        "thr": thr_np,
        "hbias": hbias_np,
    }
    if use_bias:
        weights["lbias"] = lbias_np

    feats_bf = feats.astype(bf16)
    idx16 = (nbr - HALF).astype(np.int16)                  # [N, K]

    in_maps = []
    for c in range(NCORES):
        v0 = c * NL
        a = idx16[v0:v0 + NL]                              # [NL, K]
        # [NL, K] -> per chunk [K, 16, 128] -> pad to [K, 16, IW] -> [16, K*IW]
        b = a.reshape(NCHUNK_N, T // 16, 16, K)
        b = b.transpose(0, 3, 2, 1)                        # [n, K, 16, 128]
        pad = np.zeros((NCHUNK_N, K, 16, IW - T // 16), np.int16)
        b = np.concatenate([b, pad], axis=3)               # [n, K, 16, IW]
        iw = b.transpose(2, 0, 1, 3).reshape(16, IDXF)
        in_maps.append({
            "fsl": np.ascontiguousarray(feats_bf[v0:v0 + NL]),
            "idxw": np.ascontiguousarray(iw),
        })
    return in_maps, weights, use_bias


def _whash(weights, use_bias, reps):
    h = hashlib.blake2b(digest_size=16)
    for k in sorted(weights):
        h.update(k.encode())
        h.update(weights[k].tobytes())
    h.update(bytes([use_bias, reps]))
    return h.hexdigest()


def _get_program(weights, use_bias, reps=1):
    key = _whash(weights, use_bias, reps)
    if key not in _PROGRAM:
        _PROGRAM[key] = _build_program(weights, use_bias, reps)
    return _PROGRAM[key]


def _get_exec(nc):
    """Build (once per program) the jitted sharded executor + device-zeros fn."""
    cached = getattr(nc, "_exec_cache", None)
    if cached is not None:
        return cached
    import jax
    import jax.numpy as jnp
    from jax.sharding import Mesh, PartitionSpec, NamedSharding
    from jax.experimental.shard_map import shard_map
    from concourse import bass2jax
    import concourse.mybir as _mb

    bass2jax.install_neuronx_cc_hook()

    pname = nc.partition_id_tensor.name if nc.partition_id_tensor else None
    in_names, out_names, out_avals = [], [], []
    for alloc in nc.m.functions[0].allocations:
        if not isinstance(alloc, _mb.MemoryLocationSet):
            continue
        name = alloc.memorylocations[0].name
        if alloc.kind == "ExternalInput":
            if name != pname:
                in_names.append(name)
        elif alloc.kind == "ExternalOutput":
            out_names.append(name)
            shape = tuple(alloc.tensor_shape)
            dtype = _mb.dt.np(alloc.dtype)
            out_avals.append(jax.core.ShapedArray(shape, dtype))
    n_params = len(in_names)
    n_outs = len(out_avals)
    all_names = in_names + out_names
    if pname is not None:
        all_names = all_names + [pname]
    donate = tuple(range(n_params, n_params + n_outs))

    def _body(*args):
        operands = list(args)
        if pname is not None:
            operands.append(bass2jax.partition_id_tensor())
        outs = bass2jax._bass_exec_p.bind(
            *operands,
            out_avals=tuple(out_avals),
            in_names=tuple(all_names),
            out_names=tuple(out_names),
            lowering_input_output_aliases=(),
            sim_require_finite=True,
            sim_require_nnan=True,
            nc=nc,
        )
        return tuple(outs)

    devices = jax.devices()[:NCORES]
    mesh = Mesh(np.asarray(devices), ("core",))
    in_specs = (PartitionSpec("core"),) * (n_params + n_outs)
    out_specs = (PartitionSpec("core"),) * n_outs
    fn = jax.jit(shard_map(_body, mesh=mesh, in_specs=in_specs,
                           out_specs=out_specs, check_rep=False),
                 donate_argnums=donate, keep_unused=True)
    sh = NamedSharding(mesh, PartitionSpec("core"))

    zshapes = [(NCORES * a.shape[0], *a.shape[1:]) for a in out_avals]
    zdtypes = [a.dtype for a in out_avals]
    zfn = jax.jit(
        lambda: tuple(jnp.zeros(s, d) for s, d in zip(zshapes, zdtypes)),
        out_shardings=tuple(sh for _ in zshapes))

    cached = (fn, zfn, sh, in_names, out_names, out_avals)
    nc._exec_cache = cached
    return cached


def _stage_inputs(in_maps, in_names, sh):
    import jax
    hosts = [np.concatenate([np.asarray(in_maps[c][name]) for c in range(NCORES)],
                            axis=0) for name in in_names]
    dev_in = jax.device_put(hosts, [sh] * len(hosts))
    for a in dev_in:
        a.block_until_ready()
    return dev_in


def _run(nc, in_maps):
    fn, zfn, sh, in_names, out_names, out_avals = _get_exec(nc)
    dev_in = _stage_inputs(in_maps, in_names, sh)
    zeros = zfn()
    res = fn(*dev_in, *zeros)
    for a in res:
        a.block_until_ready()
    return [{name: np.asarray(res[i]).reshape(NCORES, *out_avals[i].shape)[c]
             for i, name in enumerate(out_names)} for c in range(NCORES)]


_HEAD_PERM = np.array([6] + list(range(6)) + list(range(7, 17)))


def _assemble(results):
    out = np.zeros((NCLS, 2 * N, 17), dtype=np.float32)
    for c in range(NCORES):
        dev = results[c]["out"]                       # [NCLS, NCH, 17, TC] bf16
        core = np.ascontiguousarray(
            dev[:, :, _HEAD_PERM].transpose(0, 1, 3, 2)
        ).reshape(NCLS, M, 17).astype(np.float32)
        v0 = c * NL
        out[:, v0:v0 + NL] = core[:, :NL]
        out[:, N + v0:N + v0 + NL] = core[:, NL:]
    return out


_PREP_KEYS = ("feats", "nbr", "Wsem", "bsem", "fo_w", "fo_g", "fo_b",
              "cls_out_w", "cls_out_g", "cls_out_b", "up_w", "up_g", "up_b",
              "fuse_w", "fuse_g", "fuse_b", "exp_w", "exp_g", "exp_b",
              "ctr_w", "reg_w", "cls_w", "cls_b", "scales")


def kernel(**inputs):
    global LAST_RESULTS
    in_maps, weights, use_bias = _prepare(**{k: inputs[k] for k in _PREP_KEYS})
    nc = _get_program(weights, use_bias)
    res = _run(nc, in_maps)
    LAST_RESULTS = res
    return _assemble(res)


def benchmark(iters=3, reps=1, **inputs):
    """Time pure device execution (inputs pre-staged). Returns (best_ns, out)."""
    import time

    in_maps, weights, use_bias = _prepare(**{k: inputs[k] for k in _PREP_KEYS})
    nc = _get_program(weights, use_bias, reps)
    fn, zfn, sh, in_names, out_names, out_avals = _get_exec(nc)
    dev_in = _stage_inputs(in_maps, in_names, sh)
    print("[bench] inputs staged", flush=True)
    times, out_arrs = [], None
    for it in range(iters + 1):
        zeros = zfn()
        for a in zeros:
            a.block_until_ready()
        t0 = time.perf_counter()
        res = fn(*dev_in, *zeros)
        for a in res:
            a.block_until_ready()
        dt = time.perf_counter() - t0
        print(f"[bench] iter {it} exec {dt*1e3:.3f} ms", flush=True)
        if it > 0:
            times.append(dt)
        out_arrs = res
    best_ns = int(min(times) * 1e9)
    results = [{name: np.asarray(out_arrs[i]).reshape(NCORES, *out_avals[i].shape)[c]
                for i, name in enumerate(out_names)} for c in range(NCORES)]
    return best_ns, _assemble(results)
